# revision 2
# baseline (speedup 1.0000x reference)
import numpy as np
import jax
import jax.numpy as jnp
from jax import lax
from jax.sharding import Mesh, PartitionSpec as P
from jax.experimental.shard_map import shard_map
from functools import partial

D = 256
NH = 8
NP = 8
HD = D // NH
LQ = 4096
H = 180
W = 180
FFN = 256
NC = 8
QS = LQ // NC          # 512 queries per core
RH = 23                # conv output rows per core (8*23 = 184 >= 180)
HWF = (RH * NC) * W    # padded flattened spatial size 33120


def _np_sigmoid(x):
    return 1.0 / (1.0 + np.exp(-x))


def _ln(x, g, b):
    m = jnp.mean(x, axis=-1, keepdims=True)
    v = jnp.mean(jnp.square(x - m), axis=-1, keepdims=True)
    return (x - m) * lax.rsqrt(v + 1e-5) * g + b


def _ffn(x, p):
    h = jax.nn.relu(x @ p["ffn_w1"].T + p["ffn_b1"])
    return jax.nn.relu(h @ p["ffn_w2"].T + p["ffn_b2"])


def _msdeform_v(q, ref, v, p):
    # q: (Lq,D) shard; ref: (Lq,2); v: (NH,HW,HD) full table (already projected)
    Lq = q.shape[0]
    off = (q @ p["off_w"].T + p["off_b"]).reshape(Lq, NH, NP, 2)
    aw = jax.nn.softmax((q @ p["aw_w"].T + p["aw_b"]).reshape(Lq, NH, NP), axis=-1)
    loc = ref[:, None, None, :] + off / jnp.array([W, H], dtype=q.dtype)
    x = loc[..., 0] * W - 0.5
    y = loc[..., 1] * H - 0.5
    x0 = jnp.floor(x)
    y0 = jnp.floor(y)
    wx = x - x0
    wy = y - y0

    def gather(yi, xi):
        valid = ((xi >= 0) & (xi < W) & (yi >= 0) & (yi < H)).astype(q.dtype)
        idx = (jnp.clip(yi, 0, H - 1) * W + jnp.clip(xi, 0, W - 1)).astype(jnp.int32)
        idx = idx.transpose(1, 0, 2).reshape(NH, Lq * NP)  # (NH, Lq*NP)
        g = jnp.take_along_axis(v, idx[..., None], axis=1).reshape(NH, Lq, NP, HD)
        return g * valid.transpose(1, 0, 2)[..., None]

    def wT(w_):
        return w_.transpose(1, 0, 2)[..., None]  # (NH,Lq,NP,1)

    samp = (gather(y0, x0) * wT((1 - wx) * (1 - wy))
            + gather(y0, x0 + 1) * wT(wx * (1 - wy))
            + gather(y0 + 1, x0) * wT((1 - wx) * wy)
            + gather(y0 + 1, x0 + 1) * wT(wx * wy))
    out = jnp.einsum('hqpd,qhp->qhd', samp, aw).reshape(Lq, D)
    return out @ p["out_w"].T + p["out_b"]


def _project_v(src_flat, p):
    # src_flat: (HW, D) -> (NH, HW, HD)
    v = (src_flat @ p["val_w"].T + p["val_b"]).reshape(-1, NH, HD)
    return v.transpose(1, 0, 2)


def _cross_block(tgt, qpos, ref, v, p):
    t2 = _ln(tgt, p["ln1_g"], p["ln1_b"])
    t2 = _msdeform_v(t2 + qpos, ref, v, p)
    tgt = tgt + t2
    return tgt + _ffn(_ln(tgt, p["ln2_g"], p["ln2_b"]), p)


def _self_block(tgt, qpos, p):
    t2 = _ln(tgt, p["ln1_g"], p["ln1_b"])
    qk = t2 + qpos
    qk_all = lax.all_gather(qk, 'x', axis=0, tiled=True)   # (LQ, D)
    v_all = lax.all_gather(t2, 'x', axis=0, tiled=True)    # (LQ, D)
    wq, wk, wv = jnp.split(p["in_w"], 3, axis=0)
    bq, bk, bv = jnp.split(p["in_b"], 3)
    q = ((qk @ wq.T + bq) * (1.0 / np.sqrt(HD))).reshape(-1, NH, HD)
    k = (qk_all @ wk.T + bk).reshape(LQ, NH, HD)
    v = (v_all @ wv.T + bv).reshape(LQ, NH, HD)
    s = jax.nn.softmax(jnp.einsum('qhd,khd->hqk', q, k), axis=-1)
    o = jnp.einsum('hqk,khd->qhd', s, v).reshape(-1, D)
    t2o = o @ p["op_w"].T + p["op_b"]
    tgt = tgt + t2o
    return tgt + _ffn(_ln(tgt, p["ln2_g"], p["ln2_b"]), p)


def _body(cam_flat, lidar_flat, conv_in, tgt0, qe_s, ref_s, prm):
    # cam_flat/lidar_flat: (HW, D) replicated; conv_in: (512, 25, 182) this core's
    # padded halo slice; tgt0/qe_s/ref_s: (QS, ...) query shard.
    v1 = _project_v(lidar_flat, prm["cross1"])
    tgt = _cross_block(tgt0, qe_s, ref_s, v1, prm["cross1"])
    v2 = _project_v(cam_flat, prm["cross2"])
    tgt = _cross_block(tgt, qe_s, ref_s, v2, prm["cross2"])

    # conv fuser on this core's row slice (halo + zero pad baked into conv_in)
    fused = lax.conv_general_dilated(
        conv_in, prm["fuser_w"], window_strides=(1, 1), padding='VALID',
        dimension_numbers=('NCHW', 'OIHW', 'NCHW'))  # (1, 256, RH, 180)
    fused = jax.nn.relu(fused * prm["fuser_gamma"][None, :, None, None]
                        + prm["fuser_beta"][None, :, None, None])
    fused_flat = fused.reshape(D, RH * W).T  # (RH*W, D)
    v3c = (fused_flat @ prm["cross3"]["val_w"].T + prm["cross3"]["val_b"])
    v3_all = lax.all_gather(v3c, 'x', axis=0, tiled=True)  # (HWF, D)
    v3 = v3_all[:H * W].reshape(-1, NH, HD).transpose(1, 0, 2)
    tgt = _cross_block(tgt, qe_s, ref_s, v3, prm["cross3"])

    tgt = _self_block(tgt, qe_s, prm["self1"])
    tgt = _self_block(tgt, qe_s, prm["self2"])
    return tgt


_COMPILED = {}


def _get_fn(mesh, prm_tree):
    key = "fn"
    if key in _COMPILED:
        return _COMPILED[key]
    prm_specs = jax.tree.map(lambda _: P(), prm_tree)
    fn = shard_map(
        _body, mesh=mesh,
        in_specs=(P(), P(), P('x', None, None, None), P('x', None),
                  P('x', None), P('x', None), prm_specs),
        out_specs=P('x', None),
        check_rep=False)
    fn = jax.jit(fn)
    _COMPILED[key] = fn
    return fn


def kernel(cam_ft, lidar_ft, params):
    cam_ft = np.asarray(cam_ft, dtype=np.float32)
    lidar_ft = np.asarray(lidar_ft, dtype=np.float32)
    p = jax.tree.map(lambda a: np.asarray(a, dtype=np.float32), params)

    # ---- host precompute (params-only quantities) ----
    qe = p["query_embed"][:, :D]
    tgt0 = p["query_embed"][:, D:]
    ref = _np_sigmoid(qe @ p["rp_w"].T + p["rp_b"])  # (LQ, 2)

    cam_flat = cam_ft.reshape(D, H * W).T.copy()      # (HW, D)
    lidar_flat = lidar_ft.reshape(D, H * W).T.copy()  # (HW, D)

    # conv input slices with halo rows and zero padding baked in:
    # core c computes output rows [RH*c, RH*c+RH); needs input rows
    # [RH*c-1, RH*c+RH+1) and one zero column on each side of x.
    src = np.concatenate([cam_ft[0], lidar_ft[0]], axis=0)  # (512, 180, 180)
    pad = np.zeros((2 * D, H + 2 * NC, W + 2), dtype=np.float32)
    pad[:, 1:H + 1, 1:W + 1] = src
    conv_in = np.stack([pad[:, RH * c:RH * c + RH + 2, :] for c in range(NC)])

    prm = {k: p[k] for k in ("fuser_w", "fuser_gamma", "fuser_beta",
                             "cross1", "cross2", "cross3", "self1", "self2")}

    mesh = Mesh(np.array(jax.devices()[:NC]), ('x',))
    fn = _get_fn(mesh, prm)
    out = fn(jnp.asarray(cam_flat), jnp.asarray(lidar_flat),
             jnp.asarray(conv_in),
             jnp.asarray(tgt0), jnp.asarray(qe), jnp.asarray(ref), prm)
    out = np.asarray(jax.device_get(out), dtype=np.float32)
    return out[None]  # (1, LQ, D)
